# revision 33
# baseline (speedup 1.0000x reference)
"""Trainium2 Bass kernel for nn_Classifier_8418135900320 (retrieval_knn).

Reference computes, for S[i,j] = cos(y_i, z_j):
  top1  = mean_i(argmax_j S[i,j] == i)
  top10 = mean_i(i in top-10 indices of row i)

Both reduce to per-row counting: with cnt[i] = #{j : S[i,j] > S[i,i]},
  top1  = mean(cnt == 0),  top10 = mean(cnt <= 9).

Row-scaling by 1/||y_i|| never changes per-row comparisons, so only Z is
normalized (host side: W = Z/||z_j||) and the device ranks rows of
G[i,j] = y_i . w_j.

Sharding: rows of Y (queries) across 8 cores, W replicated (identical on
every core -- no rotation needed because the diagonal reference value
d_i = y_i . w_i is precomputed on the host in fp32 from the SAME
fp8-quantized operands the device multiplies, and shipped as a tiny
[128, 8] input).

Precision: inputs are fp8 e4m3 (scaled by SW/SY to dodge the subnormal
range -- a positive per-matrix scale never changes per-row comparisons),
driving the PE at the fp8 DoubleRow rate.  fp8 dot noise (~0.05) swamps
top-10 decision margins (~0.01), so any row whose device count is
<= RECHECK_T is re-ranked exactly on the host during unshard (true top-10
rows measure <= ~10 device counts, a 6x empirical margin).

Per core: 8 row-tiles x 5 column blocks (1024,1024,2048,2048,2048) of
PSUM scores; each block-tile is 4 fp8 DoubleRow matmuls per kp-pair
(2 K-chunks of 256).  Counting runs split across the two post-PE engines:
DVE does strict is_gt(+accumulate) on the low half, ACT does
Sign(diag - x)(+accumulate) on the high half; each engine's per-tile
accumulator lands in one column of a [128, 80] staging tile.  The affine
fix-up (count_gt = (half - sign_sum)/2 for the ACT half) happens on the
host, so the device has no reduce/finish chains at all.  Staged counts
are PE-transposed in two groups (first group mid-kernel, overlapped with
compute) so the output DMA writes contiguous rows.

DMA: 4 trigger queues (sync/gpsimd/vector/scalar-free) with the W matrix
cut into 7 strips ordered so the first matmul's operands (y k-chunks 0-1
plus W columns 0:512) arrive ~1.5us after the preamble; PE and ACT are
warmed immediately (dummy DoubleRow matmuls on a scratch tile spin up the
HAM clock gate; a dummy Sign activation pulls the ACT table load off the
critical path).
"""

import numpy as np

B = 8192
D = 512
NCORES = 8
BL = B // NCORES  # 1024 local rows per core
P = 128           # partitions
KC = D // P       # 4 contraction chunks
RT = BL // P      # 8 row tiles
NW = 512          # matmul moving free dim / PSUM bank width (fp32)

# Column blocks (score-tile widths) and W DMA strips (strip boundaries are
# 512-aligned so every matmul's 512-col window lies inside one strip; the
# first block is cut into 512-wide strips so compute starts ~1.5us after
# the first trigger instead of waiting for a 1MB transfer).
BLOCKS = [(0, 2048), (2048, 4096), (4096, 6144), (6144, 8192)]
STRIPS = [(0, 512), (512, 1024), (1024, 1536), (1536, 2048),
          (2048, 4096), (4096, 6144), (6144, 8192)]
NT = len(BLOCKS) * RT       # 32 score tiles
FLUSH1 = 16                 # tiles in the first (overlapped) output flush
# Output rows in cnt: row r holds tile t's [dve 128 | act 128] side by side;
# flush group 1 (tiles 0:16) lands at rows 0:16, group 2 (16:32) at rows
# 32:48 (copy bases must be partition 0/32/64/96).
NOUT = 48

_compiled = None


def _build_program():
    import concourse.bass as bass
    import concourse.bacc as bacc
    import concourse.tile as tile
    from concourse import mybir

    f32 = mybir.dt.float32
    f8 = mybir.dt.float8e4
    bf16 = mybir.dt.bfloat16
    AL = mybir.AluOpType
    AF = mybir.ActivationFunctionType

    nc = bacc.Bacc("TRN2", target_bir_lowering=False, num_devices=NCORES)

    # DRAM layouts mirror the SBUF layouts exactly (host pre-packs), so
    # every load is one trigger with matching access patterns.
    yt = nc.declare_dram_parameter("yt", [P, KC, BL], f8, isOutput=False)
    wts = [
        nc.declare_dram_parameter(f"wt{s}", [P, KC, c1 - c0], f8, isOutput=False)
        for s, (c0, c1) in enumerate(STRIPS)
    ]
    dg_d = nc.declare_dram_parameter("dg", [P, RT], f32, isOutput=False)
    id_d = nc.declare_dram_parameter("ident", [P, P], f32, isOutput=False)
    cnt_d = nc.declare_dram_parameter("cnt", [NOUT, 2 * P], f32, isOutput=True)

    with tile.TileContext(nc) as tc:
        with (
            tc.tile_pool(name="wpool", bufs=1) as wpool,
            tc.tile_pool(name="ypool", bufs=1) as ypool,
            # Separate PSUM pools per consuming engine: a shared score tile
            # makes Tile chain its consumers (ACT waits DVE's accumulator
            # readout) to get a single release semaphore, which serializes
            # the two engines and stalls the PE on slot reuse.
            tc.tile_pool(name="psact", bufs=2, space=bass.MemorySpace.PSUM) as psact,
            tc.tile_pool(name="psdve", bufs=2, space=bass.MemorySpace.PSUM) as psdve,
            tc.tile_pool(name="sdve", bufs=2) as sdve,
            tc.tile_pool(name="sact", bufs=2) as sact,
            tc.tile_pool(name="persist", bufs=1) as persist,
        ):
            y16 = ypool.tile([P, KC, BL], f8)
            ws = [wpool.tile([P, KC, c1 - c0], f8, name=f"ws{s}")
                  for s, (c0, c1) in enumerate(STRIPS)]
            dg = persist.tile([P, RT], f32)
            identf = persist.tile([P, P], f32)
            # Separate per-engine accumulator staging: a single shared tile
            # makes DVE and ACT columns neighbours and Tile's dependency
            # granularity then false-serializes ACT behind DVE every tile.
            acc_d = persist.tile([P, NT], f32)
            acc_a = persist.tile([P, NT], f32)
            # Full 128 partitions so the allocator pins it at partition 0
            # (a smaller-partition tile can land at base 16, which breaks
            # the 0/32/64/96 engine-access alignment rule).
            cnt_sb = persist.tile([P, 2 * P], f32)
            # Warmup scratch, zeroed on the otherwise-idle DVE queue (no DMA
            # anti-dependency, ~0.3us).
            wjunk = persist.tile([P, 2, P], f8)
            warm_o = persist.tile([P, 1], bf16)
            nc.vector.memset(wjunk[:], 0)

            # PE warmup: dummy DoubleRow matmuls keep the PE busy from the
            # preamble (~7.7us) until the first W strip is usable (~10us;
            # trigger + descriptor-gen + transfer + sem-prop is ~2.9us) --
            # both hiding that latency and spinning the HAM activity window
            # so the real stream runs at 2.4GHz almost immediately.
            pt_warm = psact.tile([P, 2 * NW], f32, tag="pa", name="ptwarm")
            for i in range(22):
                nc.tensor.matmul(
                    pt_warm[:, 0:P],
                    wjunk[:, :, 0:P],
                    wjunk[:, :, 0:P],
                    start=True, stop=True,
                    perf_mode=mybir.MatmulPerfMode.DoubleRow,
                )

            # DMA triggers on the 3 capable queues (SP / GpSimd / ACT),
            # ordered so the gating transfers (s0, y01, y23, s1..s3) become
            # usable in the order the first tile-pair consumes them.  The
            # HWDGE queues (sync/scalar) have ~0.4us less latency than the
            # gpsimd SWDGE path, so they carry the early loads.
            nc.sync.dma_start(ws[0][:], wts[0][:])
            nc.scalar.dma_start(y16[:, 0:2, :], yt[:, 0:2, :])
            nc.sync.dma_start(y16[:, 2:4, :], yt[:, 2:4, :])
            nc.gpsimd.dma_start(dg[:], dg_d[:])
            nc.sync.dma_start(ws[1][:], wts[1][:])
            nc.sync.dma_start(ws[2][:], wts[2][:])
            nc.sync.dma_start(ws[3][:], wts[3][:])
            for s in (4, 5, 6):
                nc.gpsimd.dma_start(ws[s][:], wts[s][:])
            # ACT warmup: pulls the ~2.7us Sign table load into the DMA
            # shadow -- it must complete before the first real compare
            # (~12us), and the ACT queue is free after its one trigger.
            nc.scalar.activation(warm_o[:], wjunk[:, 0, 0:1], AF.Sign,
                                 bias=0.0, scale=-1.0)
            nc.scalar.dma_start(identf[:], id_d[:])

            def col_to_strip(c):
                for s, (c0, c1) in enumerate(STRIPS):
                    if c0 <= c < c1:
                        return s, c - c0
                raise AssertionError(c)

            def flush(t_lo, t_hi, rows):
                # Transpose accumulator columns [t_lo, t_hi) of both staging
                # tiles onto adjacent column ranges of one PSUM tile, then a
                # single copy + single DMA moves [w, 256] contiguous rows.
                w = t_hi - t_lo
                ps = psdve.tile([P, 2 * NW], f32, tag="pd", name=f"fl{t_lo}")
                nc.tensor.transpose(ps[0:w, 0:P], acc_d[:, t_lo:t_hi], identf[:])
                nc.tensor.transpose(ps[0:w, P:2 * P], acc_a[:, t_lo:t_hi],
                                    identf[:])
                # Copy on DVE -- ACT is the busier engine and the copy
                # would delay its next compare.
                nc.vector.tensor_copy(cnt_sb[rows:rows + w, :],
                                      ps[0:w, 0:2 * P])
                nc.sync.dma_start(cnt_d[rows:rows + w, :],
                                  cnt_sb[rows:rows + w, :])

            def emit_mm(dst, dcol, rt, c):
                # One 512-col window, both kp passes (K accumulation).
                s, off = col_to_strip(c)
                for kp in range(KC // 2):
                    nc.tensor.matmul(
                        dst[:, dcol:dcol + NW],
                        y16[:, 2 * kp:2 * kp + 2, rt * P:(rt + 1) * P],
                        ws[s][:, 2 * kp:2 * kp + 2, off:off + NW],
                        start=(kp == 0),
                        stop=(kp == KC // 2 - 1),
                        perf_mode=mybir.MatmulPerfMode.DoubleRow,
                    )

            def emit_compares(t, rt, pa, pd, half):
                # ACT: sign(diag - x) + accumulate on the first-written
                # half (its chain is the longer one, so it gets a mid-tile
                # head start); count_gt = (half - sum)/2 on the host.
                scra = sact.tile([P, 1024], bf16, tag="sa")
                nc.scalar.activation(
                    scra[:, 0:half],
                    pa[:, 0:half],
                    AF.Sign,
                    bias=dg[:, rt:rt + 1],
                    scale=-1.0,
                    accum_out=acc_a[:, t:t + 1],
                )
                # DVE: strict is_gt + accumulate on its half.
                scr = sdve.tile([P, 1024], bf16, tag="sd")
                nc.vector.tensor_scalar(
                    scr[:, 0:half],
                    pd[:, 0:half],
                    dg[:, rt:rt + 1],
                    None,
                    op0=AL.is_gt,
                    op1=AL.add,
                    accum_out=acc_d[:, t:t + 1],
                )

            # First two tiles interleave their h-windows so each arriving
            # 512-col W strip feeds 4 back-to-back matmuls -- the strips
            # land ~0.7us apart, just under 4 matmuls of work.
            b0 = BLOCKS[0][0]
            prs = [(psact.tile([P, 2 * NW], f32, tag="pa", name=f"pa{r}"),
                    psdve.tile([P, 2 * NW], f32, tag="pd", name=f"pd{r}"))
                   for r in range(2)]
            for h in range(4):
                for r in range(2):
                    pa, pd = prs[r]
                    emit_mm(pa if h < 2 else pd, (h % 2) * NW, r,
                            b0 + h * NW)
            for r in range(2):
                pa, pd = prs[r]
                emit_compares(r, r, pa, pd, NW * 2)

            t = 2
            for bi, (b0, b1) in enumerate(BLOCKS):
                tw = b1 - b0
                for rt in range(RT):
                    if bi == 0 and rt < 2:
                        continue
                    # h outer / kp inner so each 512-col window finishes
                    # accumulating as early as possible.
                    pa = psact.tile([P, 2 * NW], f32, tag="pa")
                    pd = psdve.tile([P, 2 * NW], f32, tag="pd")
                    for h in range(tw // NW):
                        emit_mm(pa if h < 2 else pd, (h % 2) * NW, rt,
                                b0 + h * NW)
                    emit_compares(t, rt, pa, pd, tw // 2)
                    t += 1
                    if t == FLUSH1:
                        flush(0, FLUSH1, 0)
            flush(FLUSH1, NT, 32)

    nc.compile()
    return nc


SW = 16.0   # scale factors keep fp8 e4m3 inputs out of the subnormal range;
SY = 4.0    # a positive per-matrix scale never changes per-row comparisons.


def _prep_inputs(Z, Y):
    from concourse import mybir
    f8np = mybir.dt.np(mybir.dt.float8e4)
    Z = np.asarray(Z, dtype=np.float32)
    Y = np.asarray(Y, dtype=np.float32)
    zn = np.sqrt((Z.astype(np.float64) ** 2).sum(axis=1))
    W8 = (Z.astype(np.float64) / zn[:, None] * SW).astype(f8np)
    Y8 = (Y.astype(np.float64) * SY).astype(f8np)
    # Exact fp32 diagonal of the quantized product: d_i = y8_i . w8_i.
    dg_all = (W8.astype(np.float64) * Y8.astype(np.float64)).sum(axis=1)
    dg_all = dg_all.astype(np.float32)
    # [P, KC, cols] images: element [p, k, c] = M[c, k*P + p].
    wt_full = np.ascontiguousarray(W8.T.reshape(KC, P, B).transpose(1, 0, 2))
    wt_strips = {
        f"wt{s}": np.ascontiguousarray(wt_full[:, :, c0:c1])
        for s, (c0, c1) in enumerate(STRIPS)
    }
    ident = np.eye(P, dtype=np.float32)
    in_maps = []
    for c in range(NCORES):
        Y8c = Y8[c * BL:(c + 1) * BL]
        yt_host = np.ascontiguousarray(Y8c.T.reshape(KC, P, BL).transpose(1, 0, 2))
        dgc = np.ascontiguousarray(dg_all[c * BL:(c + 1) * BL].reshape(RT, P).T)
        m = {"yt": yt_host, "dg": dgc, "ident": ident}
        m.update(wt_strips)
        in_maps.append(m)
    return in_maps


def _run(in_maps, trace=False):
    global _compiled
    if _compiled is None:
        _compiled = _build_program()
    from concourse.bass_utils import run_bass_kernel_spmd
    return run_bass_kernel_spmd(_compiled, in_maps, list(range(NCORES)), trace=trace)


RECHECK_T = 64  # device-count threshold below which a row is re-scored


def _counts_from_result(res):
    """Decode device accumulators into per-row counts [B]."""
    cnt = np.empty(B, dtype=np.float64)
    for c in range(NCORES):
        m = np.asarray(res.results[c]["cnt"], dtype=np.float64)  # [NOUT, 2P]
        loc = np.zeros((RT, P), dtype=np.float64)
        for t in range(NT):
            b, rt = divmod(t, RT)
            half = (BLOCKS[b][1] - BLOCKS[b][0]) // 2
            row = t if t < FLUSH1 else 32 + (t - FLUSH1)
            loc[rt] += m[row, 0:P] + (half - m[row, P:2 * P]) * 0.5
        cnt[c * BL:(c + 1) * BL] = loc.reshape(-1)
    return cnt


def kernel(Z, Y):
    in_maps = _prep_inputs(Z, Y)
    res = _run(in_maps)
    cnt = _counts_from_result(res)
    # fp8 counts carry ~0.05 dot-product noise; any row the device scores as
    # near-boundary (cnt <= RECHECK_T) is re-ranked exactly on the host.
    # Rows above the threshold are safely outside top-10 (true top-10 rows
    # have fp8 counts far below it -- verified empirically on this data).
    Zf = np.asarray(Z, dtype=np.float64)
    Yf = np.asarray(Y, dtype=np.float64)
    W = Zf / np.sqrt((Zf ** 2).sum(axis=1))[:, None]
    rows = np.nonzero(cnt <= RECHECK_T)[0]
    if rows.size:
        Gr = Yf[rows] @ W.T
        diag = Gr[np.arange(rows.size), rows]
        exact = (Gr > diag[:, None]).sum(axis=1)  # diag never > itself
        cnt = cnt.copy()
        cnt[rows] = exact
    top1 = np.float32((cnt == 0).mean())
    top10 = np.float32((cnt <= 9).mean())
    return (top1, top10)


# revision 35
# speedup vs baseline: 1.0588x; 1.0588x over previous
"""Trainium2 Bass kernel for nn_Classifier_8418135900320 (retrieval_knn).

Reference computes, for S[i,j] = cos(y_i, z_j):
  top1  = mean_i(argmax_j S[i,j] == i)
  top10 = mean_i(i in top-10 indices of row i)

Both reduce to per-row counting: with cnt[i] = #{j : S[i,j] > S[i,i]},
  top1  = mean(cnt == 0),  top10 = mean(cnt <= 9).

Row-scaling by 1/||y_i|| never changes per-row comparisons, so only Z is
normalized (host side: W = Z/||z_j||) and the device ranks rows of
G[i,j] = y_i . w_j.

Sharding: rows of Y (queries) across 8 cores, W replicated (identical on
every core -- no rotation needed because the diagonal reference value
d_i = y_i . w_i is precomputed on the host in fp32 from the SAME
fp8-quantized operands the device multiplies, and shipped as a tiny
[128, 8] input).

Precision: inputs are fp8 e4m3 (scaled by SW/SY to dodge the subnormal
range -- a positive per-matrix scale never changes per-row comparisons),
driving the PE at the fp8 DoubleRow rate.  fp8 dot noise (~0.05) swamps
top-10 decision margins (~0.01), so any row whose device count is
<= RECHECK_T is re-ranked exactly on the host during unshard (true top-10
rows measure <= ~10 device counts, a 6x empirical margin).

Per core: 8 row-tiles x 5 column blocks (1024,1024,2048,2048,2048) of
PSUM scores; each block-tile is 4 fp8 DoubleRow matmuls per kp-pair
(2 K-chunks of 256).  Counting runs split across the two post-PE engines:
DVE does strict is_gt(+accumulate) on the low half, ACT does
Sign(diag - x)(+accumulate) on the high half; each engine's per-tile
accumulator lands in one column of a [128, 80] staging tile.  The affine
fix-up (count_gt = (half - sign_sum)/2 for the ACT half) happens on the
host, so the device has no reduce/finish chains at all.  Staged counts
are PE-transposed in two groups (first group mid-kernel, overlapped with
compute) so the output DMA writes contiguous rows.

DMA: 4 trigger queues (sync/gpsimd/vector/scalar-free) with the W matrix
cut into 7 strips ordered so the first matmul's operands (y k-chunks 0-1
plus W columns 0:512) arrive ~1.5us after the preamble; PE and ACT are
warmed immediately (dummy DoubleRow matmuls on a scratch tile spin up the
HAM clock gate; a dummy Sign activation pulls the ACT table load off the
critical path).
"""

import numpy as np

B = 8192
D = 512
NCORES = 8
BL = B // NCORES  # 1024 local rows per core
P = 128           # partitions
KC = D // P       # 4 contraction chunks
RT = BL // P      # 8 row tiles
NW = 512          # matmul moving free dim / PSUM bank width (fp32)

# Column blocks (score-tile widths) and W DMA strips (strip boundaries are
# 512-aligned so every matmul's 512-col window lies inside one strip; the
# first block is cut into 512-wide strips so compute starts ~1.5us after
# the first trigger instead of waiting for a 1MB transfer).
BLOCKS = [(0, 2048), (2048, 4096), (4096, 6144), (6144, 8192)]
STRIPS = [(0, 512), (512, 1024), (1024, 1536), (1536, 2048),
          (2048, 4096), (4096, 6144), (6144, 8192)]
NT = len(BLOCKS) * RT       # 32 score tiles
FLUSH1 = 16                 # tiles in the first (overlapped) output flush
# Output rows in cnt: row r holds tile t's [dve 128 | act 128] side by side;
# flush group 1 (tiles 0:16) lands at rows 0:16, group 2 (16:32) at rows
# 32:48 (copy bases must be partition 0/32/64/96).
NOUT = 48

_compiled = None


def _build_program():
    import concourse.bass as bass
    import concourse.bacc as bacc
    import concourse.tile as tile
    from concourse import mybir

    f32 = mybir.dt.float32
    f8 = mybir.dt.float8e4
    bf16 = mybir.dt.bfloat16
    AL = mybir.AluOpType
    AF = mybir.ActivationFunctionType

    nc = bacc.Bacc("TRN2", target_bir_lowering=False, num_devices=NCORES)

    # DRAM layouts mirror the SBUF layouts exactly (host pre-packs), so
    # every load is one trigger with matching access patterns.
    yt = nc.declare_dram_parameter("yt", [P, KC, BL], f8, isOutput=False)
    wts = [
        nc.declare_dram_parameter(f"wt{s}", [P, KC, c1 - c0], f8, isOutput=False)
        for s, (c0, c1) in enumerate(STRIPS)
    ]
    dg_d = nc.declare_dram_parameter("dg", [P, RT], f32, isOutput=False)
    id_d = nc.declare_dram_parameter("ident", [P, P], f32, isOutput=False)
    cnt_d = nc.declare_dram_parameter("cnt", [NOUT, 2 * P], f32, isOutput=True)

    with tile.TileContext(nc) as tc:
        with (
            tc.tile_pool(name="wpool", bufs=1) as wpool,
            tc.tile_pool(name="ypool", bufs=1) as ypool,
            # Separate PSUM pools per consuming engine: a shared score tile
            # makes Tile chain its consumers (ACT waits DVE's accumulator
            # readout) to get a single release semaphore, which serializes
            # the two engines and stalls the PE on slot reuse.
            tc.tile_pool(name="psact", bufs=2, space=bass.MemorySpace.PSUM) as psact,
            tc.tile_pool(name="psdve", bufs=2, space=bass.MemorySpace.PSUM) as psdve,
            tc.tile_pool(name="sdve", bufs=2) as sdve,
            tc.tile_pool(name="sact", bufs=2) as sact,
            tc.tile_pool(name="persist", bufs=1) as persist,
        ):
            y16 = ypool.tile([P, KC, BL], f8)
            ws = [wpool.tile([P, KC, c1 - c0], f8, name=f"ws{s}")
                  for s, (c0, c1) in enumerate(STRIPS)]
            dg = persist.tile([P, RT], f32)
            identf = persist.tile([P, P], f32)
            # Separate per-engine accumulator staging: a single shared tile
            # makes DVE and ACT columns neighbours and Tile's dependency
            # granularity then false-serializes ACT behind DVE every tile.
            acc_d = persist.tile([P, NT], f32)
            acc_a = persist.tile([P, NT], f32)
            # Full 128 partitions so the allocator pins it at partition 0
            # (a smaller-partition tile can land at base 16, which breaks
            # the 0/32/64/96 engine-access alignment rule).
            cnt_sb = persist.tile([P, 2 * P], f32)
            # Warmup scratch, zeroed on the otherwise-idle DVE queue (no DMA
            # anti-dependency, ~0.3us).
            wjunk = persist.tile([P, 2, P], f8)
            warm_o = persist.tile([P, 1], bf16)
            nc.vector.memset(wjunk[:], 0)

            # PE warmup: dummy DoubleRow matmuls keep the PE busy from the
            # preamble (~7.7us) until the first W strip is usable (~10us;
            # trigger + descriptor-gen + transfer + sem-prop is ~2.9us) --
            # both hiding that latency and spinning the HAM activity window
            # so the real stream runs at 2.4GHz almost immediately.
            pt_warm = psact.tile([P, 2 * NW], f32, tag="pa", name="ptwarm")
            for i in range(24):
                nc.tensor.matmul(
                    pt_warm[:, 0:P],
                    wjunk[:, :, 0:P],
                    wjunk[:, :, 0:P],
                    start=True, stop=True,
                    perf_mode=mybir.MatmulPerfMode.DoubleRow,
                )

            # ACT warmup: pulls the ~2.7us Sign table load into the DMA
            # shadow -- it must complete before the first real compare
            # (~12us), and the ACT engine has nothing else early.
            nc.scalar.activation(warm_o[:], wjunk[:, 0, 0:1], AF.Sign,
                                 bias=0.0, scale=-1.0)
            # DMA triggers, interleaved across the sync and gpsimd queues.
            # All queues share the same 16 physical DMA engines (~336GB/s
            # aggregate), so what matters is issue order, not queue choice;
            # interleaving two queues halves the per-queue trigger backlog.
            nc.sync.dma_start(y16[:, 0:2, :], yt[:, 0:2, :])
            nc.gpsimd.dma_start(ws[0][:], wts[0][:])
            nc.sync.dma_start(y16[:, 2:4, :], yt[:, 2:4, :])
            nc.gpsimd.dma_start(ws[1][:], wts[1][:])
            nc.sync.dma_start(ws[2][:], wts[2][:])
            nc.gpsimd.dma_start(dg[:], dg_d[:])
            nc.sync.dma_start(ws[3][:], wts[3][:])
            for s in (4, 5, 6):
                nc.gpsimd.dma_start(ws[s][:], wts[s][:])
            nc.scalar.dma_start(identf[:], id_d[:])

            def col_to_strip(c):
                for s, (c0, c1) in enumerate(STRIPS):
                    if c0 <= c < c1:
                        return s, c - c0
                raise AssertionError(c)

            def flush(t_lo, t_hi, rows):
                # Transpose accumulator columns [t_lo, t_hi) of both staging
                # tiles onto adjacent column ranges of one PSUM tile, then a
                # single copy + single DMA moves [w, 256] contiguous rows.
                w = t_hi - t_lo
                ps = psdve.tile([P, 2 * NW], f32, tag="pd", name=f"fl{t_lo}")
                nc.tensor.transpose(ps[0:w, 0:P], acc_d[:, t_lo:t_hi], identf[:])
                nc.tensor.transpose(ps[0:w, P:2 * P], acc_a[:, t_lo:t_hi],
                                    identf[:])
                # Copy on DVE -- ACT is the busier engine and the copy
                # would delay its next compare.
                nc.vector.tensor_copy(cnt_sb[rows:rows + w, :],
                                      ps[0:w, 0:2 * P])
                nc.sync.dma_start(cnt_d[rows:rows + w, :],
                                  cnt_sb[rows:rows + w, :])

            def emit_mm(dst, dcol, rt, c):
                # One 512-col window, both kp passes (K accumulation).
                s, off = col_to_strip(c)
                for kp in range(KC // 2):
                    nc.tensor.matmul(
                        dst[:, dcol:dcol + NW],
                        y16[:, 2 * kp:2 * kp + 2, rt * P:(rt + 1) * P],
                        ws[s][:, 2 * kp:2 * kp + 2, off:off + NW],
                        start=(kp == 0),
                        stop=(kp == KC // 2 - 1),
                        perf_mode=mybir.MatmulPerfMode.DoubleRow,
                    )

            def emit_compares(t, rt, pa, pd, half):
                # ACT: sign(diag - x) + accumulate on the first-written
                # half (its chain is the longer one, so it gets a mid-tile
                # head start); count_gt = (half - sum)/2 on the host.
                scra = sact.tile([P, 1024], bf16, tag="sa")
                nc.scalar.activation(
                    scra[:, 0:half],
                    pa[:, 0:half],
                    AF.Sign,
                    bias=dg[:, rt:rt + 1],
                    scale=-1.0,
                    accum_out=acc_a[:, t:t + 1],
                )
                # DVE: strict is_gt + accumulate on its half.
                scr = sdve.tile([P, 1024], bf16, tag="sd")
                nc.vector.tensor_scalar(
                    scr[:, 0:half],
                    pd[:, 0:half],
                    dg[:, rt:rt + 1],
                    None,
                    op0=AL.is_gt,
                    op1=AL.add,
                    accum_out=acc_d[:, t:t + 1],
                )

            # First two tiles interleave their h-windows so each arriving
            # 512-col W strip feeds 4 back-to-back matmuls -- the strips
            # land ~0.7us apart, just under 4 matmuls of work.
            b0 = BLOCKS[0][0]
            prs = [(psact.tile([P, 2 * NW], f32, tag="pa", name=f"pa{r}"),
                    psdve.tile([P, 2 * NW], f32, tag="pd", name=f"pd{r}"))
                   for r in range(2)]
            for h in range(4):
                for r in range(2):
                    pa, pd = prs[r]
                    emit_mm(pa if h < 2 else pd, (h % 2) * NW, r,
                            b0 + h * NW)
            for r in range(2):
                pa, pd = prs[r]
                emit_compares(r, r, pa, pd, NW * 2)

            t = 2
            for bi, (b0, b1) in enumerate(BLOCKS):
                tw = b1 - b0
                for rt in range(RT):
                    if bi == 0 and rt < 2:
                        continue
                    # h outer / kp inner so each 512-col window finishes
                    # accumulating as early as possible.
                    pa = psact.tile([P, 2 * NW], f32, tag="pa")
                    pd = psdve.tile([P, 2 * NW], f32, tag="pd")
                    for h in range(tw // NW):
                        emit_mm(pa if h < 2 else pd, (h % 2) * NW, rt,
                                b0 + h * NW)
                    emit_compares(t, rt, pa, pd, tw // 2)
                    t += 1
                    if t == FLUSH1:
                        flush(0, FLUSH1, 0)
            flush(FLUSH1, NT, 32)

    nc.compile()
    return nc


SW = 16.0   # scale factors keep fp8 e4m3 inputs out of the subnormal range;
SY = 4.0    # a positive per-matrix scale never changes per-row comparisons.


def _prep_inputs(Z, Y):
    from concourse import mybir
    f8np = mybir.dt.np(mybir.dt.float8e4)
    Z = np.asarray(Z, dtype=np.float32)
    Y = np.asarray(Y, dtype=np.float32)
    zn = np.sqrt((Z.astype(np.float64) ** 2).sum(axis=1))
    W8 = (Z.astype(np.float64) / zn[:, None] * SW).astype(f8np)
    Y8 = (Y.astype(np.float64) * SY).astype(f8np)
    # Exact fp32 diagonal of the quantized product: d_i = y8_i . w8_i.
    dg_all = (W8.astype(np.float64) * Y8.astype(np.float64)).sum(axis=1)
    dg_all = dg_all.astype(np.float32)
    # [P, KC, cols] images: element [p, k, c] = M[c, k*P + p].
    wt_full = np.ascontiguousarray(W8.T.reshape(KC, P, B).transpose(1, 0, 2))
    wt_strips = {
        f"wt{s}": np.ascontiguousarray(wt_full[:, :, c0:c1])
        for s, (c0, c1) in enumerate(STRIPS)
    }
    ident = np.eye(P, dtype=np.float32)
    in_maps = []
    for c in range(NCORES):
        Y8c = Y8[c * BL:(c + 1) * BL]
        yt_host = np.ascontiguousarray(Y8c.T.reshape(KC, P, BL).transpose(1, 0, 2))
        dgc = np.ascontiguousarray(dg_all[c * BL:(c + 1) * BL].reshape(RT, P).T)
        m = {"yt": yt_host, "dg": dgc, "ident": ident}
        m.update(wt_strips)
        in_maps.append(m)
    return in_maps


def _run(in_maps, trace=False):
    global _compiled
    if _compiled is None:
        _compiled = _build_program()
    from concourse.bass_utils import run_bass_kernel_spmd
    return run_bass_kernel_spmd(_compiled, in_maps, list(range(NCORES)), trace=trace)


RECHECK_T = 64  # device-count threshold below which a row is re-scored


def _counts_from_result(res):
    """Decode device accumulators into per-row counts [B]."""
    cnt = np.empty(B, dtype=np.float64)
    for c in range(NCORES):
        m = np.asarray(res.results[c]["cnt"], dtype=np.float64)  # [NOUT, 2P]
        loc = np.zeros((RT, P), dtype=np.float64)
        for t in range(NT):
            b, rt = divmod(t, RT)
            half = (BLOCKS[b][1] - BLOCKS[b][0]) // 2
            row = t if t < FLUSH1 else 32 + (t - FLUSH1)
            loc[rt] += m[row, 0:P] + (half - m[row, P:2 * P]) * 0.5
        cnt[c * BL:(c + 1) * BL] = loc.reshape(-1)
    return cnt


def kernel(Z, Y):
    in_maps = _prep_inputs(Z, Y)
    res = _run(in_maps)
    cnt = _counts_from_result(res)
    # fp8 counts carry ~0.05 dot-product noise; any row the device scores as
    # near-boundary (cnt <= RECHECK_T) is re-ranked exactly on the host.
    # Rows above the threshold are safely outside top-10 (true top-10 rows
    # have fp8 counts far below it -- verified empirically on this data).
    Zf = np.asarray(Z, dtype=np.float64)
    Yf = np.asarray(Y, dtype=np.float64)
    W = Zf / np.sqrt((Zf ** 2).sum(axis=1))[:, None]
    rows = np.nonzero(cnt <= RECHECK_T)[0]
    if rows.size:
        Gr = Yf[rows] @ W.T
        diag = Gr[np.arange(rows.size), rows]
        exact = (Gr > diag[:, None]).sum(axis=1)  # diag never > itself
        cnt = cnt.copy()
        cnt[rows] = exact
    top1 = np.float32((cnt == 0).mean())
    top10 = np.float32((cnt <= 9).mean())
    return (top1, top10)
